# revision 11
# baseline (speedup 1.0000x reference)
"""Trainium2 Bass kernel for the dual-GRU + MLP regression head problem.

Problem (hardcoded shapes):
  ru, en: [2048, 128, 130] f32
  Two PyTorch-convention GRUs (H=10), take last hidden state of each,
  concat -> [B, 20], Linear(20->20), Linear(20->1) -> [B, 1].

Strategy:
  - Pure data parallelism: batch 2048 sharded 8 ways (256 per core).
  - Host pre-transposes x to feature-major [128feat, T, 256batch] per GRU
    (features 128..129 + bias ones-row go in a small "lo" tensor).
  - Folded gate layout, free dim = batch within group (128), partitions =
    20*group + 10*gru + unit within a gate block.  Per step two PSUM tiles:
      RZ tile [104, 128]: r-gates at rows 0..39, z-gates at rows 64..103
      N  tile [104, 128]: gh_n + bhh_n at rows 0..39, xi_n + bih_n at 64..103
    (HW quadrant rule: any >32-partition operand window must start at 0/64.)
  - h state: SBUF rows 20*group + 10*gru + unit (0..39), row 64 = const 1.0
    (bias row).  h is the matmul moving operand -> no transposes anywhere.
"""

import sys

import numpy as np

sys.path.insert(0, "/opt/trn_rl_repo")

import concourse.mybir as mybir  # noqa: E402
import concourse.tile as tile  # noqa: E402
from concourse import bacc, bass_utils  # noqa: E402

B, T, I, H, D1 = 2048, 128, 130, 10, 20
NCORES = 8
BL = B // NCORES  # 256 batch per core
BG = BL // 2  # 128 batch per group
IH = 128  # "hi" features done by the big matmul
NLO = 5  # lo rows: ru128, ru129, en128, en129, ones
TB = 16  # timesteps per x-stream DMA block
LA = 2  # projection lookahead (steps ahead of recurrence)
F32 = mybir.dt.float32

_PROGRAM_CACHE = {}


def _build_weight_mats(ru_Wih, ru_Whh, ru_bih, ru_bhh,
                       en_Wih, en_Whh, en_bih, en_bhh, W1, b1, W2, b2):
    Wih = [np.asarray(ru_Wih, np.float32), np.asarray(en_Wih, np.float32)]
    Whh = [np.asarray(ru_Whh, np.float32), np.asarray(en_Whh, np.float32)]
    bih = [np.asarray(ru_bih, np.float32), np.asarray(en_bih, np.float32)]
    bhh = [np.asarray(ru_bhh, np.float32), np.asarray(en_bhh, np.float32)]

    w = {}
    # hi projection: per (gru, group): rz matrix [128, 104] and n matrix [128, 40]
    for x in range(2):
        for g in range(2):
            mrz = np.zeros((IH, 104), np.float32)
            mn = np.zeros((IH, 40), np.float32)
            c = 20 * g + 10 * x
            mrz[:, c:c + H] = Wih[x][0:10, :IH].T          # r gates
            mrz[:, 64 + c:64 + c + H] = Wih[x][10:20, :IH].T  # z gates
            mn[:, c:c + H] = Wih[x][20:30, :IH].T          # n gates
            nm = "ru" if x == 0 else "en"
            w[f"w_hi_{nm}_g{g}_rz"] = mrz
            w[f"w_hi_{nm}_g{g}_n"] = mn
    # lo projection (features 128..129 + biases): per group
    for g in range(2):
        mrz = np.zeros((NLO, 104), np.float32)
        mn = np.zeros((NLO, 40), np.float32)
        for x in range(2):
            c = 20 * g + 10 * x
            for lf in range(2):
                mrz[2 * x + lf, c:c + H] = Wih[x][0:10, IH + lf]
                mrz[2 * x + lf, 64 + c:64 + c + H] = Wih[x][10:20, IH + lf]
                mn[2 * x + lf, c:c + H] = Wih[x][20:30, IH + lf]
            # ones row: r/z carry bih+bhh; n carries bih only (bhh_n must
            # stay inside the r-gated term).
            mrz[4, c:c + H] = bih[x][0:10] + bhh[x][0:10]
            mrz[4, 64 + c:64 + c + H] = bih[x][10:20] + bhh[x][10:20]
            mn[4, c:c + H] = bih[x][20:30]
        w[f"w_lo_g{g}_rz"] = mrz
        w[f"w_lo_g{g}_n"] = mn
    # recurrent r/z weights: [40, 104]; rows = h layout (20g+10x+u).
    m = np.zeros((40, 104), np.float32)
    for g in range(2):
        for x in range(2):
            r0 = 20 * g + 10 * x
            m[r0:r0 + H, r0:r0 + H] = Whh[x][0:10, :].T
            m[r0:r0 + H, 64 + r0:64 + r0 + H] = Whh[x][10:20, :].T
    w["w_hh_rz"] = m
    # recurrent n weights (+bhh_n on ones row 64) -> N tile rows 0..39.
    m = np.zeros((65, 40), np.float32)
    for g in range(2):
        for x in range(2):
            r0 = 20 * g + 10 * x
            m[r0:r0 + H, r0:r0 + H] = Whh[x][20:30, :].T
            m[64, r0:r0 + H] = bhh[x][20:30]
    w["w_hh_n"] = m
    # MLP layer 1: per batch-group lhsT reading h directly (ones row at 64).
    for g in range(2):
        m = np.zeros((65, D1), np.float32)
        m[20 * g:20 * g + 20, :] = np.asarray(W1, np.float32).T
        m[64, :] = np.asarray(b1, np.float32)
        w[f"w_mlp1_g{g}"] = m
    m = np.zeros((33, 1), np.float32)
    m[:20, 0] = np.asarray(W2, np.float32)[0]
    m[32, 0] = np.asarray(b2, np.float32)[0]
    w["w_mlp2"] = m
    return w


def _build_per_core_inputs(ru, en):
    per_core = []
    for c in range(NCORES):
        d = {}
        for name, x in (("ru", ru), ("en", en)):
            xc = x[c * BL:(c + 1) * BL]  # [256, T, 130]
            d[f"{name}_hi"] = np.ascontiguousarray(
                xc.transpose(2, 1, 0)[:IH]).astype(np.float32, copy=False)
        xlo = np.empty((NLO, T * BL), np.float32)
        for x_i, xc in enumerate((ru[c * BL:(c + 1) * BL], en[c * BL:(c + 1) * BL])):
            for lf in range(2):
                xlo[2 * x_i + lf] = np.ascontiguousarray(
                    xc[:, :, IH + lf].T).reshape(-1)  # t-major [T*256]
        xlo[4] = 1.0
        d["xlo"] = xlo
        per_core.append(d)
    return per_core


_WSPECS = (
    ("w_hi_ru_g0_rz", [IH, 104]), ("w_hi_ru_g0_n", [IH, 40]),
    ("w_hi_ru_g1_rz", [IH, 104]), ("w_hi_ru_g1_n", [IH, 40]),
    ("w_hi_en_g0_rz", [IH, 104]), ("w_hi_en_g0_n", [IH, 40]),
    ("w_hi_en_g1_rz", [IH, 104]), ("w_hi_en_g1_n", [IH, 40]),
    ("w_lo_g0_rz", [NLO, 104]), ("w_lo_g0_n", [NLO, 40]),
    ("w_lo_g1_rz", [NLO, 104]), ("w_lo_g1_n", [NLO, 40]),
    ("w_hh_rz", [40, 104]), ("w_hh_n", [65, 40]),
    ("w_mlp1_g0", [65, D1]), ("w_mlp1_g1", [65, D1]),
    ("w_mlp2", [33, 1]),
)


def _build_program(n_steps=T, n_repeats=1):
    key = (n_steps, n_repeats)
    if key in _PROGRAM_CACHE:
        return _PROGRAM_CACHE[key]

    nc = bacc.Bacc("TRN2", target_bir_lowering=False, debug=False,
                   num_devices=NCORES)

    dram = {}
    dram["ru_hi"] = nc.dram_tensor("ru_hi", [IH, T, BL], F32, kind="ExternalInput")
    dram["en_hi"] = nc.dram_tensor("en_hi", [IH, T, BL], F32, kind="ExternalInput")
    dram["xlo"] = nc.dram_tensor("xlo", [NLO, T * BL], F32, kind="ExternalInput")
    for nm, shp in _WSPECS:
        dram[nm] = nc.dram_tensor(nm, shp, F32, kind="ExternalInput")
    out_dram = nc.dram_tensor("out", [1, BL], F32, kind="ExternalOutput")

    with tile.TileContext(nc) as tc:
        with (
            tc.tile_pool(name="wpool", bufs=1) as wpool,
            tc.tile_pool(name="xru", bufs=2) as xru_pool,
            tc.tile_pool(name="xen", bufs=2) as xen_pool,
            tc.tile_pool(name="xlo", bufs=2) as xlo_pool,
            tc.tile_pool(name="hpool", bufs=1) as hpool,
            tc.tile_pool(name="sg", bufs=2) as sig_pool,
            tc.tile_pool(name="tmp", bufs=2) as tmp_pool,
            tc.tile_pool(name="mlp", bufs=1) as mlp_pool,
            tc.tile_pool(name="psrz", bufs=3, space="PSUM") as ps_rz,
            tc.tile_pool(name="psn", bufs=3, space="PSUM") as ps_n,
            tc.tile_pool(name="psmlp", bufs=1, space="PSUM") as ps_mlp,
        ):
            # ---- load weights into SBUF ----
            wsb = {}
            for nm, shp in _WSPECS:
                t_ = wpool.tile([128, shp[1]], F32, tag=nm)
                nc.sync.dma_start(t_[0:shp[0], :], dram[nm].ap())
                wsb[nm] = t_

            # ---- h state (double buffered); row 64 = ones ----
            h_a = hpool.tile([128, BG], F32, tag="h_a")
            h_b = hpool.tile([128, BG], F32, tag="h_b")
            for h_ in (h_a, h_b):
                nc.vector.memset(h_[0:65, :], 0.0)
                nc.vector.memset(h_[64:65, :], 1.0)

            Sig = mybir.ActivationFunctionType.Sigmoid
            Tanh = mybir.ActivationFunctionType.Tanh

            xtiles = {}
            slots = {}

            def emit_proj(t):
                blk = t // TB
                if t % TB == 0:
                    xt_ru = xru_pool.tile([IH, TB * BL], F32, tag="xr")
                    xt_en = xen_pool.tile([IH, TB * BL], F32, tag="xe")
                    xt_lo = xlo_pool.tile([NLO, TB * BL], F32, tag="xl")
                    sl = slice(blk * TB, (blk + 1) * TB)
                    nc.sync.dma_start(
                        xt_ru[:], dram["ru_hi"].ap()[:, sl, :].rearrange("p t b -> p (t b)"))
                    nc.sync.dma_start(
                        xt_en[:], dram["en_hi"].ap()[:, sl, :].rearrange("p t b -> p (t b)"))
                    nc.sync.dma_start(
                        xt_lo[:], dram["xlo"].ap()[:, blk * TB * BL:(blk + 1) * TB * BL])
                    xtiles[blk] = (xt_ru, xt_en, xt_lo)
                xt_ru, xt_en, xt_lo = xtiles[blk]
                prz = ps_rz.tile([104, BG], F32, tag="prz")
                pn = ps_n.tile([104, BG], F32, tag="pn")
                slots[t] = (prz, pn)
                off = (t % TB) * BL
                first = True
                for g in range(2):
                    for nm, xt, kk in (("ru", xt_ru, IH), ("en", xt_en, IH),
                                       ("lo", xt_lo, NLO)):
                        rhs = xt[0:kk, off + g * BG: off + g * BG + BG]
                        key = f"w_{nm}_g{g}" if nm == "lo" else f"w_hi_{nm}_g{g}"
                        key = f"w_lo_g{g}" if nm == "lo" else key
                        nc.tensor.matmul(prz[:], wsb[f"{key}_rz"][0:kk, :], rhs,
                                         start=first, stop=False)
                        nc.tensor.matmul(pn[64:104, :], wsb[f"{key}_n"][0:kk, :], rhs,
                                         start=first, stop=(nm == "lo" and g == 1))
                        first = False

            def emit_step(t):
                h_cur = h_a if t % 2 == 0 else h_b
                h_next = h_b if t % 2 == 0 else h_a
                prz, pn = slots.pop(t)
                nc.tensor.matmul(prz[:], wsb["w_hh_rz"][0:40, :], h_cur[0:40, :],
                                 start=False, stop=True)
                nc.tensor.matmul(pn[0:40, :], wsb["w_hh_n"][0:65, :], h_cur[0:65, :],
                                 start=True, stop=True)
                sig_r = sig_pool.tile([40, BG], F32, tag="sig_r")
                nc.scalar.activation(sig_r[:], prz[0:40, :], Sig)
                sig_z = sig_pool.tile([40, BG], F32, tag="sig_z")
                nc.scalar.activation(sig_z[:], prz[64:104, :], Sig)
                nr = tmp_pool.tile([40, BG], F32, tag="nr")
                nc.vector.tensor_mul(nr[:], sig_r[:], pn[0:40, :])
                npre = tmp_pool.tile([40, BG], F32, tag="npre")
                nc.vector.tensor_add(npre[:], nr[:], pn[64:104, :])
                n_t = tmp_pool.tile([40, BG], F32, tag="n")
                nc.scalar.activation(n_t[:], npre[:], Tanh)
                d_t = tmp_pool.tile([40, BG], F32, tag="d")
                nc.vector.tensor_sub(d_t[:], h_cur[0:40, :], n_t[:])
                zd = tmp_pool.tile([40, BG], F32, tag="zd")
                nc.vector.tensor_mul(zd[:], sig_z[:], d_t[:])
                nc.vector.tensor_add(h_next[0:40, :], n_t[:], zd[:])

            def emit_body():
                xtiles.clear()
                for step in range(n_steps + LA):
                    if step < n_steps:
                        emit_proj(step)
                    if step >= LA:
                        emit_step(step - LA)

            if n_repeats == 1:
                emit_body()
            else:
                with tc.For_i(0, n_repeats, 1):
                    emit_body()

            # ---- MLP head ----
            h_last = h_a if n_steps % 2 == 0 else h_b
            ps1 = ps_mlp.tile([128, BL], F32, tag="ps1")
            for g in range(2):
                nc.tensor.matmul(ps1[0:D1, g * BG:(g + 1) * BG],
                                 wsb[f"w_mlp1_g{g}"][0:65, :], h_last[0:65, :],
                                 start=True, stop=True)
            h1s = mlp_pool.tile([128, BL], F32, tag="h1s")
            nc.vector.memset(h1s[0:33, :], 0.0)
            nc.vector.memset(h1s[32:33, :], 1.0)
            nc.vector.tensor_copy(h1s[0:20, :], ps1[0:D1, :])
            ps2 = ps_mlp.tile([128, BL], F32, tag="ps2")
            nc.tensor.matmul(ps2[0:1, :], wsb["w_mlp2"][0:33, :], h1s[0:33, :],
                             start=True, stop=True)
            outs = mlp_pool.tile([1, BL], F32, tag="outs")
            nc.vector.tensor_copy(outs[:], ps2[0:1, :])
            nc.sync.dma_start(out_dram.ap(), outs[:])

    nc.compile()
    _PROGRAM_CACHE[key] = nc
    return nc


def kernel(**inputs):
    ru = np.asarray(inputs["ru"], np.float32)
    en = np.asarray(inputs["en"], np.float32)
    w = _build_weight_mats(
        inputs["ru_Wih"], inputs["ru_Whh"], inputs["ru_bih"], inputs["ru_bhh"],
        inputs["en_Wih"], inputs["en_Whh"], inputs["en_bih"], inputs["en_bhh"],
        inputs["W1"], inputs["b1"], inputs["W2"], inputs["b2"])
    per_core = _build_per_core_inputs(ru, en)

    nc = _build_program()
    in_maps = []
    for c in range(NCORES):
        m = dict(per_core[c])
        m.update(w)
        in_maps.append(m)

    res = bass_utils.run_bass_kernel_spmd(nc, in_maps, core_ids=list(range(NCORES)))
    out = np.empty((B, 1), np.float32)
    for c in range(NCORES):
        out[c * BL:(c + 1) * BL, 0] = np.asarray(res.results[c]["out"]).reshape(BL)
    return out
